# revision 24
# baseline (speedup 1.0000x reference)
"""Trainium2 Bass kernel for nn_DepthwiseRREUp.

Op: depthwise conv_transpose2d with kernel=stride=2 (non-overlapping
2x2 upsampling), filters are per-(channel, group) 90-degree rotations
of a per-channel 2x2 kernel.

  x:  [B=8, C=256, G=4, H=64, W=64] f32
  dw: [C=256, 1, 2, 2] f32
  out[b, c, g, 2i+di, 2j+dj] = x[b, c, g, i, j] * rot90(dw[c, 0], g)[di, dj]

Sharding: pure data-parallel over batch, one batch element per core
(B == n_cores == 8), no communication.

The op is HBM-bandwidth bound (per core: read 16.8 MB of x, write 67.1 MB
of out in f32 — the f32 baseline sat exactly at the ~360 GB/s per-core DMA
roofline). Both streams are carried in bfloat16 instead (rel err ~8e-3,
well inside the 2e-2 gate), halving traffic: 8.4 MB in + 33.6 MB out
= 42 MB -> ~113-123 us depending on machine state.

Empirical roofline study (2026-08-10, loop-marginal timing on the
trn2.8x1 slice = 8 physical cores of one chip, 4 shared HBM domains):
  read-only probe   20.0 us -> 420 GB/s/core (~840 GB/s/domain)
  write-only probe  95.7 us -> 351 GB/s/core (~702 GB/s/domain),
                    identical for every queue split (s/sp/sa/spa)
  no-compute probe (both streams) == full kernel time -> engines fully
                    hidden; the kernel IS the DMA stream.
  full kernel       ~119-123 us vs ~116 us additive floor; the ~3%
                    residual is read/write mix loss on the shared HBM
                    domains. Not recovered by: same-queue FIFO phase
                    separation (in_eng="s"), deep x prefetch (xbufs=2),
                    out chunks 1/2/4/8, out queue splits sp/sa/spa,
                    in_eng a/s/p, dense-block DRAM write layout
                    (seq_out=True: per-DMA descriptors land sequentially
                    in DRAM — no effect on write-only or full kernel),
                    zero-SWDGE routing (in_eng="a" + out_eng="sa", all
                    DMA on the two HWDGE rings, GpSimd idle — null, so
                    the Q7 descriptor-ring/DVE port lock is not the
                    residual either).
  packed_in=True (x as [P, n_tiles*hw], one 8.4 MB read DMA/iter) is a
  measured 15% REGRESSION (135-143 us): the single giant x tile
  serializes load->compute across loop iterations; per-tile loads are
  required for fine-grained overlap. Kept as a flag for reference.
  Chunk size under the sp ring alternation (queue depth held at 96 KB):
  chunks=2 (2 MB DMAs) == chunks=4 == noise; chunks=8 (512 KB DMAs) is
  a REAL +17% regression (140.9 us) — don't shrink write DMAs below
  1 MB on dual rings.
Traffic is the floor: bf16 is the smallest dtype passing the 2e-2
max-rel-err gate — fp8e4m3 x gives ~100% max rel err (subnormal
underflow on the small-magnitude tail of x*f), measured on host.
So ~119 us (+-4% machine noise) is the hardware floor for this op; do
not expect scheduling changes to move it.

Per-core device layout:
  x    [M=1024, 4096]  bf16 (M = C*G channel-groups on partitions, H*W free)
  f    [128, 32]       f32 host-pretransposed rotated filters
  out  [M, 16384]      bf16 (interleaved H*2 x W*2 per channel-group)

Schedule: all 8 x-tiles are prefetched up front on the gpsimd (SWDGE)
queue while the out stream alternates between the SP (HWDGE) and gpsimd
(SWDGE) queues (out_eng="sp": two descriptor generators, marginally
smoother write stream — best cumulative median across A/B batteries);
per 128-row tile the four (di, dj) planes are per-partition-scalar
multiplies on DVE/ACT written strided (row stride 256, col stride 2)
into the interleaved SBUF buffer, so every DMA descriptor on both sides
is a fully contiguous >=8 KB per-partition run.
"""

import contextlib

import numpy as np
import ml_dtypes

import concourse.bacc as bacc
import concourse.mybir as mybir
from concourse import bass_utils
from concourse.tile import TileContext

# Problem constants (hardcoded per harness contract).
B, C, G, H, W = 8, 256, 4, 64, 64
K = 2
M = C * G
HW = H * W
OUT_HW = HW * K * K
P = 128
N_CORES = 8

CHUNKS = 4  # row-chunks per partition-tile (out DMA = 128 x 8KB descriptors)
ENGINES = ("v", "a", "v", "a")  # engine per (di*K+dj) plane: DVE / ACT
SEQ_OUT = False  # out DRAM layout: [tile*chunk, P, chunk] dense blocks
PACKED_IN = False  # x DRAM layout [P, n_tiles*hw]: one 8.4 MB read DMA/iter
BF16 = mybir.dt.bfloat16


def build_bass(m=M, h=H, w=W, chunks=CHUNKS, engines=ENGINES, obufs=12, reps=1,
               loop_n=None, in_eng="p", out_eng="sp", xbufs=1, prefetch=True,
               compute=True, do_in=True, do_out=True, seq_out=SEQ_OUT,
               packed_in=PACKED_IN):
    """Build the per-core Bass module. reps>1 (unrolled) or loop_n>1
    (hardware For_i loop) repeat the pipeline over the same input/output
    for marginal-time HW benchmarking."""
    hw = h * w
    out_hw = hw * K * K
    n_tiles = m // P
    rows = h // chunks
    chunk_out = out_hw // chunks

    nc = bacc.Bacc("TRN2", target_bir_lowering=False)
    if packed_in:
        # [P, n_tiles*hw]: column block t holds channel-groups t*P+p at
        # partition p — the whole per-core x loads in ONE dma_start.
        x = nc.dram_tensor("x", [P, (m // P) * hw], BF16, kind="ExternalInput")
    else:
        x = nc.dram_tensor("x", [m, hw], BF16, kind="ExternalInput")
    # f is host-pretransposed to [P, n_tiles*4]: column t*4+k holds filter
    # element k for channel-group t*P + p. Loads in ONE contiguous dma.
    f = nc.dram_tensor(
        "f", [P, K * K * n_tiles], mybir.dt.float32, kind="ExternalInput"
    )
    # seq_out: [tile*chunk, P, chunk_out] layout — every out-DMA's 128
    # per-partition descriptors land back-to-back in one dense DRAM block
    # (host permutes back). Default: [m, out_hw] row-major.
    if seq_out:
        out = nc.dram_tensor(
            "out", [(m // P) * chunks * P, out_hw // chunks], BF16,
            kind="ExternalOutput",
        )
    else:
        out = nc.dram_tensor("out", [m, out_hw], BF16, kind="ExternalOutput")
    xap, oap = x[:], out[:]
    fview = f[:]

    def eng(nc, key):
        return {"v": nc.vector, "a": nc.scalar, "p": nc.gpsimd, "s": nc.sync}[key]

    with TileContext(nc) as tc:
        with (
            tc.tile_pool(name="fpool", bufs=1) as fpool,
            tc.tile_pool(name="xpool", bufs=xbufs) as xpool,
            tc.tile_pool(name="opool", bufs=obufs) as opool,
        ):
            ft = fpool.tile([P, K * K * n_tiles], mybir.dt.float32)
            nc.sync.dma_start(out=ft, in_=fview)
            loop_ctx = tc.For_i(0, loop_n, 1) if loop_n else contextlib.nullcontext()
            with loop_ctx:
              for _rep in range(reps):
                xts = {}
                xall = None
                if packed_in:
                    xall = xpool.tile([P, n_tiles * hw], BF16, name="xall")
                    if do_in:
                        eng(nc, in_eng).dma_start(out=xall, in_=xap)
                    xallv = xall.rearrange(
                        "p (t i j) -> p t i j", t=n_tiles, j=w
                    )
                elif prefetch:
                    for t in range(n_tiles):
                        xts[t] = xpool.tile([P, hw], BF16, name=f"xt{t}")
                        if do_in:
                            eng(nc, in_eng).dma_start(
                                out=xts[t], in_=xap[t * P : (t + 1) * P, :]
                            )
                for t in range(n_tiles):
                    if packed_in:
                        xv = xallv[:, t, :, :]
                    else:
                        if prefetch:
                            xt = xts[t]
                        else:
                            xt = xpool.tile([P, hw], BF16)
                            if do_in:
                                eng(nc, in_eng).dma_start(
                                    out=xt, in_=xap[t * P : (t + 1) * P, :]
                                )
                        xv = xt.rearrange("p (i j) -> p i j", j=w)
                    for ch in range(chunks):
                        ot = opool.tile([P, chunk_out], BF16)
                        # [p, i, di, j, dj] view of the interleaved output chunk
                        ov = ot.rearrange("p (i a j b) -> p i a j b", a=K, j=w, b=K)
                        xi = xv[:, ch * rows : (ch + 1) * rows, :]
                        if not compute:
                            # DMA-roofline probe: single cheap touch so the
                            # out tile has a writer, then stream it out.
                            src = xi[:, 0:1, 0:1] if do_in else ft[:, 0:1]
                            nc.vector.tensor_scalar_mul(
                                ot[:, 0:1], src, ft[:, 0:1]
                            )
                        for di in range(K if compute else 0):
                            for dj in range(K):
                                plane = di * K + dj
                                dst = ov[:, :, di, :, dj]
                                scal = ft[:, t * K * K + plane : t * K * K + plane + 1]
                                ekey = engines[plane]
                                if ekey == "a":
                                    nc.scalar.activation(
                                        dst,
                                        xi,
                                        mybir.ActivationFunctionType.Copy,
                                        scale=scal,
                                    )
                                else:
                                    eng(nc, ekey).tensor_scalar_mul(dst, xi, scal)
                        if do_out:
                            oe = out_eng[(t * chunks + ch) % len(out_eng)]
                            if seq_out:
                                blk = t * chunks + ch
                                dst = oap[blk * P : (blk + 1) * P, :]
                            else:
                                dst = oap[
                                    t * P : (t + 1) * P,
                                    ch * chunk_out : (ch + 1) * chunk_out,
                                ]
                            eng(nc, oe).dma_start(out=dst, in_=ot)
    return nc


_NC_CACHE = {}


def _get_nc():
    if "nc" not in _NC_CACHE:
        nc = build_bass()
        nc.finalize()
        _NC_CACHE["nc"] = nc
    return _NC_CACHE["nc"]


def _build_filters_np(dw):
    # Mirrors reference._build_filters exactly (pure index permutation).
    rot = np.stack(
        [np.rot90(dw, k=i, axes=(-2, -1)) for i in range(G)], axis=1
    )  # [C, G, 1, K, K]
    return np.ascontiguousarray(rot).reshape(C * G, K * K).astype(np.float32)


def _transpose_filters(fm, m=M):
    # [m, 4] -> [P, (m//P)*4] matching the device-side f layout
    n_tiles = m // P
    return np.ascontiguousarray(
        fm.reshape(n_tiles, P, K * K).transpose(1, 0, 2).reshape(P, n_tiles * K * K)
    )


def make_in_maps(x, dw):
    x = np.asarray(x, dtype=np.float32)
    dw = np.ascontiguousarray(dw, dtype=np.float32)
    fm = _transpose_filters(_build_filters_np(dw))  # [P, 32]
    xs = np.ascontiguousarray(x.reshape(B, M, HW)).astype(ml_dtypes.bfloat16)
    if PACKED_IN:
        xs = np.ascontiguousarray(
            xs.reshape(B, M // P, P, HW).transpose(0, 2, 1, 3)
        ).reshape(B, P, (M // P) * HW)
    return [{"x": xs[b], "f": fm} for b in range(B)]


def kernel(x, dw):
    nc = _get_nc()
    in_maps = make_in_maps(x, dw)
    res = bass_utils.run_bass_kernel_spmd(nc, in_maps, core_ids=list(range(N_CORES)))
    out = np.stack([res.results[b]["out"] for b in range(B)], axis=0)
    if SEQ_OUT:
        # undo the dense-block DRAM layout: [n_tiles*chunks*P, chunk] ->
        # [M, OUT_HW]
        n_tiles, chunk = M // P, OUT_HW // CHUNKS
        out = (
            out.reshape(B, n_tiles, CHUNKS, P, chunk)
            .transpose(0, 1, 3, 2, 4)
            .reshape(B, M, OUT_HW)
        )
    return out.astype(np.float32).reshape(B, C, G, H * K, W * K)

